# revision 17
# baseline (speedup 1.0000x reference)
"""CRF forward (log-partition) on 8 Trainium2 NeuronCores.

Linear-domain scaled forward algorithm, data-parallel over the batch.

Math: the reference computes, per lane b,
    alpha_0 = onehot-ish(START);  for t < len_b:
    alpha_{t+1}[i] = u_t[i] + logsumexp_j(alpha_t[j] + tr[i, j])
    logZ = logsumexp_i(alpha_len[i] + tr[END, i])
In probability space (p = exp(alpha)) each step is
    p_{t+1} = (E @ p_t) * exp(u_t),   E = exp(tr)
a tiny [64,64] matmul plus an elementwise multiply — ideal for the PE
(stationary weights) + vector engine. Per-lane sequence lengths and the
initial state are folded into a host-prepped, right-aligned log-unary
tensor with one extra "reset" tag, so the device runs one uniform
unconditional 512-step chain for all lanes:
  - warmup steps (t < T-len-1): unary rows = NEG (exp -> 0), reset row = 0
    (exp -> 1): the reset lane carries r=1, real tags stay dead.
  - injection step t* = T-len-1: unary rows = 0, reset row = NEG: the
    matrix column for the reset tag injects onehot(START); r dies.
  - real steps: the lane's actual unaries, shifted by -ln(kappa) per step
    to keep p magnitudes centered in f32 range (measured drift stays
    within e^[-20, 10]); tr[END, :] is added at the final step.
The device streams the 512-step chain; the final state p_T [65, 128] is
DMA'd out and logZ = ln(sum_j p_T[j]) + len * ln(kappa) applied on host.
"""

import os
import sys

import numpy as np

for _p in ("/opt/trn_rl_repo", "/root/.axon_site/_ro/trn_rl_repo"):
    if os.path.isdir(_p) and _p not in sys.path:
        sys.path.append(_p)

import concourse.bacc as bacc
import concourse.bass as bass
import concourse.tile as tile
from concourse import mybir
from concourse.bass_utils import run_bass_kernel_spmd

T = 512
N = 64  # tags
NA = N + 1  # + reset tag
BL = 128  # batch lanes per core
NCORES = 8
START_IDX = 1
END_IDX = 2
NEG = -100.0  # exp(NEG) == 0 in f32 up to a ~1e-44 residue that the math kills
LNK = 5.113338285898717  # mean per-step log-growth of the partition mass
GRP = 8  # timesteps per DMA/exp tile
F32 = mybir.dt.float32


def _build_program(trace: bool = False):
    nc = bacc.Bacc("TRN2", target_bir_lowering=False, debug=False)
    up_d = nc.dram_tensor("up", [NA, T, BL], F32, kind="ExternalInput")
    # w (stationary matrix) and p0 (initial state) fused into one tensor so
    # the first matmul depends on a single DMA semaphore (PE HW allows only
    # one sync-wait per matmul).
    init_d = nc.dram_tensor("init", [NA, NA + BL], F32, kind="ExternalInput")
    out_d = nc.dram_tensor("out", [NA, BL], F32, kind="ExternalOutput")

    with tile.TileContext(nc) as tc:
        with (
            tc.tile_pool(name="singles", bufs=1) as singles,
            tc.tile_pool(name="up", bufs=4) as up_pool,
            tc.tile_pool(name="e", bufs=4) as e_pool,
            tc.tile_pool(name="p", bufs=3) as p_pool,
            tc.tile_pool(name="z", bufs=4, space="PSUM") as z_pool,
        ):
            init_sb = singles.tile([NA, NA + BL], F32)
            nc.sync.dma_start(out=init_sb, in_=init_d[:, :])
            w_sb = init_sb[:, 0:NA]
            p_cur = init_sb[:, NA : NA + BL]

            for g in range(T // GRP):
                up_sb = up_pool.tile([NA, GRP, BL], F32)
                nc.sync.dma_start(
                    out=up_sb, in_=up_d[:, g * GRP : (g + 1) * GRP, :]
                )
                e_sb = e_pool.tile([NA, GRP, BL], F32)
                nc.scalar.activation(e_sb, up_sb, mybir.ActivationFunctionType.Exp)
                for k in range(GRP):
                    z = z_pool.tile([NA, BL], F32)
                    nc.tensor.matmul(z, w_sb, p_cur, start=True, stop=True)
                    p_new = p_pool.tile([NA, BL], F32, tag="p")
                    nc.vector.tensor_mul(p_new, z, e_sb[:, k, :])
                    p_cur = p_new

            nc.sync.dma_start(out=out_d[:, :], in_=p_cur)
    nc.compile()
    return nc


def _build_core_inputs(u_core: np.ndarray, len_core: np.ndarray, tr: np.ndarray):
    """u_core [BL, T, N] f32, len_core [BL] -> up [NA, T, BL], p0 [NA, BL]."""
    up = np.full((NA, T, BL), NEG, dtype=np.float32)
    p0 = np.zeros((NA, BL), dtype=np.float32)
    for b in range(BL):
        length = int(len_core[b])
        tstar = T - length - 1
        if length == T:
            p0[START_IDX, b] = 1.0
        else:
            p0[N, b] = 1.0
            up[N, :tstar, b] = 0.0
            up[:N, tstar, b] = 0.0
        up[:N, tstar + 1 :, b] = u_core[b, :length, :].T - LNK
    up[:N, T - 1, :] += tr[END_IDX][:, None]
    return up, p0


def _build_w(tr: np.ndarray) -> np.ndarray:
    w = np.zeros((NA, NA), dtype=np.float32)
    w[:N, :N] = np.exp(tr.astype(np.float32)).T  # lhsT[j, i] = exp(tr[i, j])
    w[N, START_IDX] = 1.0  # injection column
    w[N, N] = 1.0  # reset lane survives (until its unary row kills it)
    return w


def kernel(unary: np.ndarray, trans: np.ndarray, lengths: np.ndarray) -> np.ndarray:
    unary = np.asarray(unary, dtype=np.float32)  # [B, T, N]
    tr = np.asarray(trans, dtype=np.float32)[0]  # [N, N]
    lens = np.asarray(lengths).astype(np.int64)  # [B]
    B = unary.shape[0]
    assert unary.shape == (B, T, N) and B == NCORES * BL

    w = _build_w(tr)
    in_maps = []
    for c in range(NCORES):
        sl = slice(c * BL, (c + 1) * BL)
        up, p0 = _build_core_inputs(unary[sl], lens[sl], tr)
        init = np.concatenate([w, p0], axis=1)  # [NA, NA + BL]
        in_maps.append({"up": up, "init": init})

    nc = _build_program()
    res = run_bass_kernel_spmd(nc, in_maps, list(range(NCORES)))
    sums = np.concatenate(
        [res.results[c]["out"].astype(np.float64).sum(axis=0) for c in range(NCORES)]
    )
    out = np.log(sums.astype(np.float64)) + lens.astype(np.float64) * LNK
    return out.astype(np.float32)


# revision 23
# speedup vs baseline: 1.4023x; 1.4023x over previous
"""CRF forward (log-partition) on 8 Trainium2 NeuronCores.

Linear-domain scaled forward algorithm, data-parallel over the batch.

Math: the reference computes, per lane b,
    alpha_0 = onehot-ish(START);  for t < len_b:
    alpha_{t+1}[i] = u_t[i] + logsumexp_j(alpha_t[j] + tr[i, j])
    logZ = logsumexp_i(alpha_len[i] + tr[END, i])
In probability space (p = exp(alpha)) each step is
    p_{t+1} = (E @ p_t) * exp(u_t),   E = exp(tr)
a tiny [64,64] matmul plus an elementwise multiply — ideal for the PE
(stationary weights) + vector engine. Per-lane sequence lengths and the
initial state are folded into a host-prepped, right-aligned log-unary
tensor with one extra "reset" tag, so the device runs one uniform
unconditional 512-step chain for all lanes:
  - warmup steps (t < T-len-1): unary rows = NEG (exp -> 0), reset row = 0
    (exp -> 1): the reset lane carries r=1, real tags stay dead.
  - injection step t* = T-len-1: unary rows = 0, reset row = NEG: the
    matrix column for the reset tag injects onehot(START); r dies.
  - real steps: the lane's actual unaries, shifted by -ln(kappa) per step
    to keep p magnitudes centered in f32 range (measured drift stays
    within e^[-20, 10]); tr[END, :] is added at the final step.
The device streams the 512-step chain; the final state p_T [65, 128] is
DMA'd out and logZ = ln(sum_j p_T[j]) + len * ln(kappa) applied on host.
"""

import os
import sys

import numpy as np

for _p in ("/opt/trn_rl_repo", "/root/.axon_site/_ro/trn_rl_repo"):
    if os.path.isdir(_p) and _p not in sys.path:
        sys.path.append(_p)

import concourse.bacc as bacc
import concourse.bass as bass
import concourse.tile as tile
from concourse import mybir
from concourse.bass_utils import run_bass_kernel_spmd

T = 512
N = 64  # tags
NA = N + 1  # + reset tag
BL = 128  # batch lanes per core
NCORES = 8
START_IDX = 1
END_IDX = 2
NEG = -100.0  # exp(NEG) == 0 in f32 up to a ~1e-44 residue that the math kills
LNK = 5.113338285898717  # mean per-step log-growth of the partition mass
GRP = 8  # timesteps per DMA/exp tile
F32 = mybir.dt.float32
F32R = mybir.dt.float32r  # single-pass PE matmul dtype (plain fp32 lowers
# to a HI/LO pass pair at ~4x the cost); ~19-bit storage is plenty here


def _build_program(trace: bool = False):
    nc = bacc.Bacc("TRN2", target_bir_lowering=False, debug=False)
    up_d = nc.dram_tensor("up", [NA, T, BL], F32, kind="ExternalInput")
    # w (stationary matrix) and p0 (initial state) fused into one tensor so
    # the first matmul depends on a single DMA semaphore (PE HW allows only
    # one sync-wait per matmul).
    init_d = nc.dram_tensor("init", [NA, NA + BL], F32R, kind="ExternalInput")
    out_d = nc.dram_tensor("out", [NA, BL], F32R, kind="ExternalOutput")

    with tile.TileContext(nc) as tc:
        with (
            tc.tile_pool(name="singles", bufs=1) as singles,
            tc.tile_pool(name="up", bufs=4) as up_pool,
            tc.tile_pool(name="e", bufs=4) as e_pool,
            tc.tile_pool(name="p", bufs=3) as p_pool,
            tc.tile_pool(name="z", bufs=4, space="PSUM") as z_pool,
        ):
            init_sb = singles.tile([NA, NA + BL], F32R)
            nc.sync.dma_start(out=init_sb, in_=init_d[:, :])
            w_sb = init_sb[:, 0:NA]
            p_cur = init_sb[:, NA : NA + BL]

            for g in range(T // GRP):
                up_sb = up_pool.tile([NA, GRP, BL], F32)
                nc.sync.dma_start(
                    out=up_sb, in_=up_d[:, g * GRP : (g + 1) * GRP, :]
                )
                e_sb = e_pool.tile([NA, GRP, BL], F32)
                nc.scalar.activation(e_sb, up_sb, mybir.ActivationFunctionType.Exp)
                for k in range(GRP):
                    z = z_pool.tile([NA, BL], F32)
                    nc.tensor.matmul(z, w_sb, p_cur, start=True, stop=True)
                    p_new = p_pool.tile([NA, BL], F32R, tag="p")
                    nc.vector.tensor_mul(p_new, z, e_sb[:, k, :])
                    p_cur = p_new

            nc.sync.dma_start(out=out_d[:, :], in_=p_cur)
    nc.compile()
    return nc


def _build_core_inputs(u_core: np.ndarray, len_core: np.ndarray, tr: np.ndarray):
    """u_core [BL, T, N] f32, len_core [BL] -> up [NA, T, BL], p0 [NA, BL]."""
    up = np.full((NA, T, BL), NEG, dtype=np.float32)
    p0 = np.zeros((NA, BL), dtype=np.float32)
    for b in range(BL):
        length = int(len_core[b])
        tstar = T - length - 1
        if length == T:
            p0[START_IDX, b] = 1.0
        else:
            p0[N, b] = 1.0
            up[N, :tstar, b] = 0.0
            up[:N, tstar, b] = 0.0
        up[:N, tstar + 1 :, b] = u_core[b, :length, :].T - LNK
    up[:N, T - 1, :] += tr[END_IDX][:, None]
    return up, p0


def _build_w(tr: np.ndarray) -> np.ndarray:
    w = np.zeros((NA, NA), dtype=np.float32)
    w[:N, :N] = np.exp(tr.astype(np.float32)).T  # lhsT[j, i] = exp(tr[i, j])
    w[N, START_IDX] = 1.0  # injection column
    w[N, N] = 1.0  # reset lane survives (until its unary row kills it)
    return w


def kernel(unary: np.ndarray, trans: np.ndarray, lengths: np.ndarray) -> np.ndarray:
    unary = np.asarray(unary, dtype=np.float32)  # [B, T, N]
    tr = np.asarray(trans, dtype=np.float32)[0]  # [N, N]
    lens = np.asarray(lengths).astype(np.int64)  # [B]
    B = unary.shape[0]
    assert unary.shape == (B, T, N) and B == NCORES * BL

    w = _build_w(tr)
    in_maps = []
    for c in range(NCORES):
        sl = slice(c * BL, (c + 1) * BL)
        up, p0 = _build_core_inputs(unary[sl], lens[sl], tr)
        init = np.concatenate([w, p0], axis=1)  # [NA, NA + BL]
        in_maps.append({"up": up, "init": init})

    nc = _build_program()
    res = run_bass_kernel_spmd(nc, in_maps, list(range(NCORES)))
    sums = np.concatenate(
        [res.results[c]["out"].astype(np.float64).sum(axis=0) for c in range(NCORES)]
    )
    out = np.log(sums.astype(np.float64)) + lens.astype(np.float64) * LNK
    return out.astype(np.float32)


# revision 24
# speedup vs baseline: 1.6440x; 1.1724x over previous
"""CRF forward (log-partition) on 8 Trainium2 NeuronCores.

Linear-domain scaled forward algorithm, data-parallel over the batch.

Math: the reference computes, per lane b,
    alpha_0 = onehot-ish(START);  for t < len_b:
    alpha_{t+1}[i] = u_t[i] + logsumexp_j(alpha_t[j] + tr[i, j])
    logZ = logsumexp_i(alpha_len[i] + tr[END, i])
In probability space (p = exp(alpha)) each step is
    p_{t+1} = (E @ p_t) * exp(u_t),   E = exp(tr)
a tiny [64,64] matmul plus an elementwise multiply — ideal for the PE
(stationary weights) + vector engine. Per-lane sequence lengths and the
initial state are folded into a host-prepped, right-aligned log-unary
tensor with one extra "reset" tag, so the device runs one uniform
unconditional 512-step chain for all lanes:
  - warmup steps (t < T-len-1): unary rows = NEG (exp -> 0), reset row = 0
    (exp -> 1): the reset lane carries r=1, real tags stay dead.
  - injection step t* = T-len-1: unary rows = 0, reset row = NEG: the
    matrix column for the reset tag injects onehot(START); r dies.
  - real steps: the lane's actual unaries, shifted by -ln(kappa) per step
    to keep p magnitudes centered in f32 range (measured drift stays
    within e^[-20, 10]); tr[END, :] is added at the final step.
The device streams the 512-step chain; the final state p_T [65, 128] is
DMA'd out and logZ = ln(sum_j p_T[j]) + len * ln(kappa) applied on host.
"""

import os
import sys

import numpy as np

for _p in ("/opt/trn_rl_repo", "/root/.axon_site/_ro/trn_rl_repo"):
    if os.path.isdir(_p) and _p not in sys.path:
        sys.path.append(_p)

import concourse.bacc as bacc
import concourse.bass as bass
import concourse.tile as tile
from concourse import mybir
from concourse.bass_utils import run_bass_kernel_spmd

T = 512
N = 64  # tags
NA = N + 1  # + reset tag
BL = 128  # batch lanes per core
NCORES = 8
START_IDX = 1
END_IDX = 2
NEG = -100.0  # exp(NEG) == 0 in f32 up to a ~1e-44 residue that the math kills
LNK = 5.113338285898717  # mean per-step log-growth of the partition mass
GRP = 8  # timesteps per DMA/exp tile
F32 = mybir.dt.float32
F32R = mybir.dt.float32r  # single-pass PE matmul dtype (plain fp32 lowers
# to a HI/LO pass pair at ~4x the cost); ~19-bit storage is plenty here


def _build_program(trace: bool = False):
    nc = bacc.Bacc("TRN2", target_bir_lowering=False, debug=False)
    up_d = nc.dram_tensor("up", [NA, T, BL], F32, kind="ExternalInput")
    # w (stationary matrix) and p0 (initial state) fused into one tensor so
    # the first matmul depends on a single DMA semaphore (PE HW allows only
    # one sync-wait per matmul).
    init_d = nc.dram_tensor("init", [NA, NA + BL], F32R, kind="ExternalInput")
    out_d = nc.dram_tensor("out", [NA, BL], F32R, kind="ExternalOutput")

    HB = BL // 2  # two independent half-chains per core so PE matmuls of one
    # chain overlap the DVE multiply of the other (the per-step serial
    # MM -> sem -> TT -> sem loop otherwise leaves both engines half idle)
    with tile.TileContext(nc) as tc:
        with (
            tc.tile_pool(name="singles", bufs=1) as singles,
            tc.tile_pool(name="up", bufs=4) as up_pool,
            tc.tile_pool(name="e", bufs=4) as e_pool,
            tc.tile_pool(name="pa", bufs=3) as p_pool_a,
            tc.tile_pool(name="pb", bufs=3) as p_pool_b,
            tc.tile_pool(name="za", bufs=3, space="PSUM") as z_pool_a,
            tc.tile_pool(name="zb", bufs=3, space="PSUM") as z_pool_b,
        ):
            init_sb = singles.tile([NA, NA + BL], F32R)
            nc.sync.dma_start(out=init_sb, in_=init_d[:, :])
            w_sb = init_sb[:, 0:NA]
            p_pools = (p_pool_a, p_pool_b)
            z_pools = (z_pool_a, z_pool_b)
            p_cur = [init_sb[:, NA + h * HB : NA + (h + 1) * HB] for h in range(2)]

            for g in range(T // GRP):
                up_sb = up_pool.tile([NA, GRP, BL], F32)
                nc.sync.dma_start(
                    out=up_sb, in_=up_d[:, g * GRP : (g + 1) * GRP, :]
                )
                e_sb = e_pool.tile([NA, GRP, BL], F32)
                nc.scalar.activation(e_sb, up_sb, mybir.ActivationFunctionType.Exp)
                for k in range(GRP):
                    for h in range(2):
                        z = z_pools[h].tile([NA, HB], F32, tag=f"z{h}")
                        nc.tensor.matmul(z, w_sb, p_cur[h], start=True, stop=True)
                        p_new = p_pools[h].tile([NA, HB], F32R, tag=f"p{h}")
                        nc.vector.tensor_mul(
                            p_new, z, e_sb[:, k, h * HB : (h + 1) * HB]
                        )
                        p_cur[h] = p_new

            for h in range(2):
                nc.sync.dma_start(
                    out=out_d[:, h * HB : (h + 1) * HB], in_=p_cur[h]
                )
    nc.compile()
    return nc


def _build_core_inputs(u_core: np.ndarray, len_core: np.ndarray, tr: np.ndarray):
    """u_core [BL, T, N] f32, len_core [BL] -> up [NA, T, BL], p0 [NA, BL]."""
    up = np.full((NA, T, BL), NEG, dtype=np.float32)
    p0 = np.zeros((NA, BL), dtype=np.float32)
    for b in range(BL):
        length = int(len_core[b])
        tstar = T - length - 1
        if length == T:
            p0[START_IDX, b] = 1.0
        else:
            p0[N, b] = 1.0
            up[N, :tstar, b] = 0.0
            up[:N, tstar, b] = 0.0
        up[:N, tstar + 1 :, b] = u_core[b, :length, :].T - LNK
    up[:N, T - 1, :] += tr[END_IDX][:, None]
    return up, p0


def _build_w(tr: np.ndarray) -> np.ndarray:
    w = np.zeros((NA, NA), dtype=np.float32)
    w[:N, :N] = np.exp(tr.astype(np.float32)).T  # lhsT[j, i] = exp(tr[i, j])
    w[N, START_IDX] = 1.0  # injection column
    w[N, N] = 1.0  # reset lane survives (until its unary row kills it)
    return w


def kernel(unary: np.ndarray, trans: np.ndarray, lengths: np.ndarray) -> np.ndarray:
    unary = np.asarray(unary, dtype=np.float32)  # [B, T, N]
    tr = np.asarray(trans, dtype=np.float32)[0]  # [N, N]
    lens = np.asarray(lengths).astype(np.int64)  # [B]
    B = unary.shape[0]
    assert unary.shape == (B, T, N) and B == NCORES * BL

    w = _build_w(tr)
    in_maps = []
    for c in range(NCORES):
        sl = slice(c * BL, (c + 1) * BL)
        up, p0 = _build_core_inputs(unary[sl], lens[sl], tr)
        init = np.concatenate([w, p0], axis=1)  # [NA, NA + BL]
        in_maps.append({"up": up, "init": init})

    nc = _build_program()
    res = run_bass_kernel_spmd(nc, in_maps, list(range(NCORES)))
    sums = np.concatenate(
        [res.results[c]["out"].astype(np.float64).sum(axis=0) for c in range(NCORES)]
    )
    out = np.log(sums.astype(np.float64)) + lens.astype(np.float64) * LNK
    return out.astype(np.float32)


# revision 26
# speedup vs baseline: 1.9458x; 1.1836x over previous
"""CRF forward (log-partition) on 8 Trainium2 NeuronCores.

Linear-domain scaled forward algorithm, data-parallel over the batch.

Math: the reference computes, per lane b,
    alpha_0 = onehot-ish(START);  for t < len_b:
    alpha_{t+1}[i] = u_t[i] + logsumexp_j(alpha_t[j] + tr[i, j])
    logZ = logsumexp_i(alpha_len[i] + tr[END, i])
In probability space (p = exp(alpha)) each step is
    p_{t+1} = (E @ p_t) * exp(u_t),   E = exp(tr)
a tiny [64,64] matmul plus an elementwise multiply — ideal for the PE
(stationary weights) + vector engine. Per-lane sequence lengths and the
initial state are folded into a host-prepped, right-aligned log-unary
tensor with one extra "reset" tag, so the device runs one uniform
unconditional 512-step chain for all lanes:
  - warmup steps (t < T-len-1): unary rows = NEG (exp -> 0), reset row = 0
    (exp -> 1): the reset lane carries r=1, real tags stay dead.
  - injection step t* = T-len-1: unary rows = 0, reset row = NEG: the
    matrix column for the reset tag injects onehot(START); r dies.
  - real steps: the lane's actual unaries, shifted by -ln(kappa) per step
    to keep p magnitudes centered in f32 range (measured drift stays
    within e^[-20, 10]); tr[END, :] is added at the final step.
The device streams the 512-step chain; the final state p_T [65, 128] is
DMA'd out and logZ = ln(sum_j p_T[j]) + len * ln(kappa) applied on host.
"""

import os
import sys

import numpy as np

for _p in ("/opt/trn_rl_repo", "/root/.axon_site/_ro/trn_rl_repo"):
    if os.path.isdir(_p) and _p not in sys.path:
        sys.path.append(_p)

import contextlib

import concourse.bacc as bacc
import concourse.bass as bass
import concourse.bass_utils as bass_utils
import concourse.tile as tile
from concourse import mybir
from concourse.bass_utils import run_bass_kernel_spmd


@contextlib.contextmanager
def _walrus_ldw_opt():
    """Compile this kernel with walrus LDWEIGHTS elision enabled.

    The stationary matrix here never changes, so the 512+ per-matmul weight
    reloads (~172 ns each, ~30% of PE time) are pure waste; ldw-opt removes
    the redundant ones. concourse pins --enable-ldw-opt=false globally, so
    swap the flag just for this kernel's compile."""
    orig = bass_utils.run_command

    def patched(argv, **kwargs):
        argv = [
            a.replace("--enable-ldw-opt=false", "--enable-ldw-opt=true")
            if isinstance(a, str)
            else a
            for a in argv
        ]
        return orig(argv, **kwargs)

    bass_utils.run_command = patched
    try:
        yield
    finally:
        bass_utils.run_command = orig

T = 512
N = 64  # tags
NA = N + 1  # + reset tag
BL = 128  # batch lanes per core
NCORES = 8
START_IDX = 1
END_IDX = 2
NEG = -100.0  # exp(NEG) == 0 in f32 up to a ~1e-44 residue that the math kills
LNK = 5.113338285898717  # mean per-step log-growth of the partition mass
GRP = 8  # timesteps per DMA/exp tile
F32 = mybir.dt.float32
F32R = mybir.dt.float32r  # single-pass PE matmul dtype (plain fp32 lowers
# to a HI/LO pass pair at ~4x the cost); ~19-bit storage is plenty here


def _build_program(trace: bool = False):
    nc = bacc.Bacc("TRN2", target_bir_lowering=False, debug=False)
    up_d = nc.dram_tensor("up", [NA, T, BL], F32, kind="ExternalInput")
    # w (stationary matrix) and p0 (initial state) fused into one tensor so
    # the first matmul depends on a single DMA semaphore (PE HW allows only
    # one sync-wait per matmul).
    init_d = nc.dram_tensor("init", [NA, NA + BL], F32R, kind="ExternalInput")
    out_d = nc.dram_tensor("out", [NA, BL], F32R, kind="ExternalOutput")

    HB = BL // 2  # two independent half-chains per core so PE matmuls of one
    # chain overlap the DVE multiply of the other (the per-step serial
    # MM -> sem -> TT -> sem loop otherwise leaves both engines half idle)
    with tile.TileContext(nc) as tc:
        with (
            tc.tile_pool(name="singles", bufs=1) as singles,
            tc.tile_pool(name="up", bufs=4) as up_pool,
            tc.tile_pool(name="e", bufs=4) as e_pool,
            tc.tile_pool(name="pa", bufs=3) as p_pool_a,
            tc.tile_pool(name="pb", bufs=3) as p_pool_b,
            tc.tile_pool(name="za", bufs=3, space="PSUM") as z_pool_a,
            tc.tile_pool(name="zb", bufs=3, space="PSUM") as z_pool_b,
        ):
            init_sb = singles.tile([NA, NA + BL], F32R)
            nc.sync.dma_start(out=init_sb, in_=init_d[:, :])
            w_sb = init_sb[:, 0:NA]
            p_pools = (p_pool_a, p_pool_b)
            z_pools = (z_pool_a, z_pool_b)
            p_cur = [init_sb[:, NA + h * HB : NA + (h + 1) * HB] for h in range(2)]

            for g in range(T // GRP):
                up_sb = up_pool.tile([NA, GRP, BL], F32)
                nc.sync.dma_start(
                    out=up_sb, in_=up_d[:, g * GRP : (g + 1) * GRP, :]
                )
                e_sb = e_pool.tile([NA, GRP, BL], F32)
                nc.scalar.activation(e_sb, up_sb, mybir.ActivationFunctionType.Exp)
                for k in range(GRP):
                    for h in range(2):
                        z = z_pools[h].tile([NA, HB], F32, tag=f"z{h}")
                        nc.tensor.matmul(z, w_sb, p_cur[h], start=True, stop=True)
                        p_new = p_pools[h].tile([NA, HB], F32R, tag=f"p{h}")
                        nc.vector.tensor_mul(
                            p_new, z, e_sb[:, k, h * HB : (h + 1) * HB]
                        )
                        p_cur[h] = p_new

            for h in range(2):
                nc.sync.dma_start(
                    out=out_d[:, h * HB : (h + 1) * HB], in_=p_cur[h]
                )
    nc.compile()
    return nc


def _build_core_inputs(u_core: np.ndarray, len_core: np.ndarray, tr: np.ndarray):
    """u_core [BL, T, N] f32, len_core [BL] -> up [NA, T, BL], p0 [NA, BL]."""
    up = np.full((NA, T, BL), NEG, dtype=np.float32)
    p0 = np.zeros((NA, BL), dtype=np.float32)
    for b in range(BL):
        length = int(len_core[b])
        tstar = T - length - 1
        if length == T:
            p0[START_IDX, b] = 1.0
        else:
            p0[N, b] = 1.0
            up[N, :tstar, b] = 0.0
            up[:N, tstar, b] = 0.0
        up[:N, tstar + 1 :, b] = u_core[b, :length, :].T - LNK
    up[:N, T - 1, :] += tr[END_IDX][:, None]
    return up, p0


def _build_w(tr: np.ndarray) -> np.ndarray:
    w = np.zeros((NA, NA), dtype=np.float32)
    w[:N, :N] = np.exp(tr.astype(np.float32)).T  # lhsT[j, i] = exp(tr[i, j])
    w[N, START_IDX] = 1.0  # injection column
    w[N, N] = 1.0  # reset lane survives (until its unary row kills it)
    return w


def kernel(unary: np.ndarray, trans: np.ndarray, lengths: np.ndarray) -> np.ndarray:
    unary = np.asarray(unary, dtype=np.float32)  # [B, T, N]
    tr = np.asarray(trans, dtype=np.float32)[0]  # [N, N]
    lens = np.asarray(lengths).astype(np.int64)  # [B]
    B = unary.shape[0]
    assert unary.shape == (B, T, N) and B == NCORES * BL

    w = _build_w(tr)
    in_maps = []
    for c in range(NCORES):
        sl = slice(c * BL, (c + 1) * BL)
        up, p0 = _build_core_inputs(unary[sl], lens[sl], tr)
        init = np.concatenate([w, p0], axis=1)  # [NA, NA + BL]
        in_maps.append({"up": up, "init": init})

    nc = _build_program()
    with _walrus_ldw_opt():
        res = run_bass_kernel_spmd(nc, in_maps, list(range(NCORES)))
    sums = np.concatenate(
        [res.results[c]["out"].astype(np.float64).sum(axis=0) for c in range(NCORES)]
    )
    out = np.log(sums.astype(np.float64)) + lens.astype(np.float64) * LNK
    return out.astype(np.float32)


# revision 28
# speedup vs baseline: 1.9468x; 1.0005x over previous
"""CRF forward (log-partition) on 8 Trainium2 NeuronCores.

Linear-domain scaled forward algorithm, data-parallel over the batch.

Math: the reference computes, per lane b,
    alpha_0 = onehot-ish(START);  for t < len_b:
    alpha_{t+1}[i] = u_t[i] + logsumexp_j(alpha_t[j] + tr[i, j])
    logZ = logsumexp_i(alpha_len[i] + tr[END, i])
In probability space (p = exp(alpha)) each step is
    p_{t+1} = (E @ p_t) * exp(u_t),   E = exp(tr)
a tiny [64,64] matmul plus an elementwise multiply — ideal for the PE
(stationary weights) + vector engine. Per-lane sequence lengths and the
initial state are folded into a host-prepped, right-aligned log-unary
tensor with one extra "reset" tag, so the device runs one uniform
unconditional 512-step chain for all lanes:
  - warmup steps (t < T-len-1): unary rows = NEG (exp -> 0), reset row = 0
    (exp -> 1): the reset lane carries r=1, real tags stay dead.
  - injection step t* = T-len-1: unary rows = 0, reset row = NEG: the
    matrix column for the reset tag injects onehot(START); r dies.
  - real steps: the lane's actual unaries, shifted by -ln(kappa) per step
    to keep p magnitudes centered in f32 range (measured drift stays
    within e^[-20, 10]); tr[END, :] is added at the final step.
The device streams the 512-step chain; the final state p_T [65, 128] is
DMA'd out and logZ = ln(sum_j p_T[j]) + len * ln(kappa) applied on host.
"""

import os
import sys

import numpy as np

for _p in ("/opt/trn_rl_repo", "/root/.axon_site/_ro/trn_rl_repo"):
    if os.path.isdir(_p) and _p not in sys.path:
        sys.path.append(_p)

import contextlib

import concourse.bacc as bacc
import concourse.bass as bass
import concourse.bass_utils as bass_utils
import concourse.tile as tile
from concourse import mybir
from concourse.bass_utils import run_bass_kernel_spmd


@contextlib.contextmanager
def _walrus_ldw_opt():
    """Compile this kernel with walrus LDWEIGHTS elision enabled.

    The stationary matrix here never changes, so the 512+ per-matmul weight
    reloads (~172 ns each, ~30% of PE time) are pure waste; ldw-opt removes
    the redundant ones. concourse pins --enable-ldw-opt=false globally, so
    swap the flag just for this kernel's compile."""
    orig = bass_utils.run_command

    def patched(argv, **kwargs):
        argv = [
            a.replace("--enable-ldw-opt=false", "--enable-ldw-opt=true")
            if isinstance(a, str)
            else a
            for a in argv
        ]
        return orig(argv, **kwargs)

    bass_utils.run_command = patched
    try:
        yield
    finally:
        bass_utils.run_command = orig

T = 512
N = 64  # tags
NA = N + 1  # + reset tag
BL = 128  # batch lanes per core
NCORES = 8
START_IDX = 1
END_IDX = 2
NEG = -100.0  # exp(NEG) == 0 in f32 up to a ~1e-44 residue that the math kills
LNK = 5.113338285898717  # mean per-step log-growth of the partition mass
GRP = 8  # timesteps per DMA/exp tile
F32 = mybir.dt.float32
F32R = mybir.dt.float32r  # single-pass PE matmul dtype (plain fp32 lowers
# to a HI/LO pass pair at ~4x the cost); ~19-bit storage is plenty here


def _build_program(trace: bool = False):
    nc = bacc.Bacc("TRN2", target_bir_lowering=False, debug=False)
    up_d = nc.dram_tensor("up", [NA, T, BL], F32, kind="ExternalInput")
    # w (stationary matrix) and p0 (initial state) fused into one tensor so
    # the first matmul depends on a single DMA semaphore (PE HW allows only
    # one sync-wait per matmul).
    init_d = nc.dram_tensor("init", [NA, NA + BL], F32R, kind="ExternalInput")
    out_d = nc.dram_tensor("out", [NA, BL], F32R, kind="ExternalOutput")

    HB = BL // 2  # two independent half-chains per core so PE matmuls of one
    # chain overlap the DVE multiply of the other (the per-step serial
    # MM -> sem -> TT -> sem loop otherwise leaves both engines half idle)
    with tile.TileContext(nc) as tc:
        with (
            tc.tile_pool(name="singles", bufs=1) as singles,
            tc.tile_pool(name="upa", bufs=6) as up_pool_a,
            tc.tile_pool(name="upb", bufs=6) as up_pool_b,
            tc.tile_pool(name="ea", bufs=6) as e_pool_a,
            tc.tile_pool(name="eb", bufs=6) as e_pool_b,
            tc.tile_pool(name="pa", bufs=4) as p_pool_a,
            tc.tile_pool(name="pb", bufs=4) as p_pool_b,
            tc.tile_pool(name="za", bufs=4, space="PSUM") as z_pool_a,
            tc.tile_pool(name="zb", bufs=4, space="PSUM") as z_pool_b,
        ):
            init_sb = singles.tile([NA, NA + BL], F32R)
            nc.sync.dma_start(out=init_sb, in_=init_d[:, :])
            w_sb = init_sb[:, 0:NA]
            p_pools = (p_pool_a, p_pool_b)
            z_pools = (z_pool_a, z_pool_b)
            p_cur = [init_sb[:, NA + h * HB : NA + (h + 1) * HB] for h in range(2)]

            up_pools = (up_pool_a, up_pool_b)
            e_pools = (e_pool_a, e_pool_b)
            for g in range(T // GRP):
                e_sbs = []
                # per-half DMA + exp so neither chain's multiply gates on the
                # other chain's unary pipeline at group boundaries
                for h in range(2):
                    up_sb = up_pools[h].tile([NA, GRP, HB], F32, tag=f"up{h}")
                    nc.sync.dma_start(
                        out=up_sb,
                        in_=up_d[:, g * GRP : (g + 1) * GRP, h * HB : (h + 1) * HB],
                    )
                    e_sb = e_pools[h].tile([NA, GRP, HB], F32, tag=f"e{h}")
                    nc.scalar.activation(
                        e_sb, up_sb, mybir.ActivationFunctionType.Exp
                    )
                    e_sbs.append(e_sb)
                for k in range(GRP):
                    for h in range(2):
                        z = z_pools[h].tile([NA, HB], F32, tag=f"z{h}")
                        nc.tensor.matmul(z, w_sb, p_cur[h], start=True, stop=True)
                        p_new = p_pools[h].tile([NA, HB], F32R, tag=f"p{h}")
                        nc.vector.tensor_mul(p_new, z, e_sbs[h][:, k, :])
                        p_cur[h] = p_new

            for h in range(2):
                nc.sync.dma_start(
                    out=out_d[:, h * HB : (h + 1) * HB], in_=p_cur[h]
                )
    nc.compile()
    return nc


def _build_core_inputs(u_core: np.ndarray, len_core: np.ndarray, tr: np.ndarray):
    """u_core [BL, T, N] f32, len_core [BL] -> up [NA, T, BL], p0 [NA, BL]."""
    up = np.full((NA, T, BL), NEG, dtype=np.float32)
    p0 = np.zeros((NA, BL), dtype=np.float32)
    for b in range(BL):
        length = int(len_core[b])
        tstar = T - length - 1
        if length == T:
            p0[START_IDX, b] = 1.0
        else:
            p0[N, b] = 1.0
            up[N, :tstar, b] = 0.0
            up[:N, tstar, b] = 0.0
        up[:N, tstar + 1 :, b] = u_core[b, :length, :].T - LNK
    up[:N, T - 1, :] += tr[END_IDX][:, None]
    return up, p0


def _build_w(tr: np.ndarray) -> np.ndarray:
    w = np.zeros((NA, NA), dtype=np.float32)
    w[:N, :N] = np.exp(tr.astype(np.float32)).T  # lhsT[j, i] = exp(tr[i, j])
    w[N, START_IDX] = 1.0  # injection column
    w[N, N] = 1.0  # reset lane survives (until its unary row kills it)
    return w


def kernel(unary: np.ndarray, trans: np.ndarray, lengths: np.ndarray) -> np.ndarray:
    unary = np.asarray(unary, dtype=np.float32)  # [B, T, N]
    tr = np.asarray(trans, dtype=np.float32)[0]  # [N, N]
    lens = np.asarray(lengths).astype(np.int64)  # [B]
    B = unary.shape[0]
    assert unary.shape == (B, T, N) and B == NCORES * BL

    w = _build_w(tr)
    in_maps = []
    for c in range(NCORES):
        sl = slice(c * BL, (c + 1) * BL)
        up, p0 = _build_core_inputs(unary[sl], lens[sl], tr)
        init = np.concatenate([w, p0], axis=1)  # [NA, NA + BL]
        in_maps.append({"up": up, "init": init})

    nc = _build_program()
    with _walrus_ldw_opt():
        res = run_bass_kernel_spmd(nc, in_maps, list(range(NCORES)))
    sums = np.concatenate(
        [res.results[c]["out"].astype(np.float64).sum(axis=0) for c in range(NCORES)]
    )
    out = np.log(sums.astype(np.float64)) + lens.astype(np.float64) * LNK
    return out.astype(np.float32)
